# revision 1
# baseline (speedup 1.0000x reference)
"""Trainium2 Bass kernel for the constant-velocity Kalman filter (T=131072, 6-dof pose).

Reformulation: the covariance/gain recursion is measurement-independent, so the
Kalman gain converges to its steady state K_ss after ~30 steps (Riccati, done on
host in fp64).  From there the filter state is a linear time-invariant recursion
  est_t = A est_{t-1} + K m_t,   A = (I - K H) F,  rho(A) = 0.618,
so est_t equals a 128-tap FIR of the measurements to below fp32 precision
(|A^64 K| ~ 2e-14).  With isotropic noise the taps are scalar per channel, and
all three outputs (pose estimate / prediction / velocity) are scalar FIR
convolutions of the 6-channel measurement stream.

Device work: each of the 8 cores owns 16384 timesteps, laid out as 128 blocks x
128 steps (block-index on the SBUF free dim, in-block time on partitions). The
convolution is two accumulating 128x128 banded-Toeplitz matmuls per output on
the tensor engine (current block + previous block), i.e. overlap-save with the
overlap handled by PSUM accumulation.  The first ~170 rows (Riccati transient)
are patched on the host with the exact sequential recursion in fp64.
"""

import numpy as np

T = 131072
NCORES = 8
TPC = T // NCORES            # timesteps per core
BLK = 128                    # in-block timesteps (partition dim)
NB = TPC // BLK              # 128 blocks per core
NTAPS = 128                  # FIR taps 0..127
RICCATI_CAP = 2048

_compiled = {}
LAST_RESULTS = None


# --------------------------------------------------------------------------- #
# host-side model math (fp64)
# --------------------------------------------------------------------------- #

def _model():
    F = np.zeros((12, 12))
    F[:6, :6] = 2 * np.eye(6)
    F[:6, 6:] = -np.eye(6)
    F[6:, :6] = np.eye(6)
    H = np.zeros((6, 12))
    H[:, :6] = np.eye(6)
    return F, H


def _riccati(Qc, Rc, P0):
    """Run the gain recursion; K_t applies at absolute row t = i + 2."""
    F, H = _model()
    Q = np.zeros((12, 12))
    Q[:6, :6] = Qc
    P = P0.copy()
    Ks = []
    conv = None
    for i in range(RICCATI_CAP):
        Pp = F @ P @ F.T + Q
        S = Pp[:6, :6] + Rc
        K = Pp[:, :6] @ np.linalg.inv(S)
        KH = np.zeros((12, 12))
        KH[:, :6] = K
        P = Pp - KH @ Pp
        Ks.append(K)
        if i > 0 and np.abs(K - Ks[-2]).max() < 1e-13 * (1.0 + np.abs(K).max()):
            conv = i
            break
    return Ks, conv


def _steady_taps(K_ss):
    """Matrix taps A^k K_ss; returns (taps[128,12,6], w0[128], w1[128], iso_ok)."""
    F, H = _model()
    A = (np.eye(12) - K_ss @ H) @ F
    taps = np.zeros((NTAPS, 12, 6))
    Ak = np.eye(12)
    for k in range(NTAPS):
        taps[k] = Ak @ K_ss
        Ak = Ak @ A
    w0 = taps[:, 0, 0].copy()
    w1 = taps[:, 6, 0].copy()
    I6 = np.eye(6)
    iso_err = max(
        np.abs(taps[:, :6, :] - w0[:, None, None] * I6).max(),
        np.abs(taps[:, 6:, :] - w1[:, None, None] * I6).max(),
    )
    tail = np.abs(taps[NTAPS - 1]).max() + np.abs(Ak).max()
    iso_ok = iso_err < 1e-9 and tail < 1e-9
    return taps, w0, w1, iso_ok


def _exact_prefix(m, Ks, t_hi):
    """Exact fp64 sequential recursion for rows 2..t_hi; returns est[t] (12,) array."""
    F, _ = _model()
    est = np.zeros((t_hi + 1, 12))
    pred = np.zeros((t_hi + 1, 12))
    est[1] = np.concatenate([m[1], m[0]])
    e = est[1].copy()
    for t in range(2, t_hi + 1):
        Kt = Ks[min(t - 2, len(Ks) - 1)]
        p = F @ e
        e = p + Kt @ (m[t] - p[:6])
        est[t] = e
        pred[t] = p
    return est, pred


def _host_fallback(m, Ks):
    """Full sequential filter on host (only for pathological covariance inputs)."""
    est, pred = _exact_prefix(m, Ks, T - 1)
    pose_est = est[:, :6].copy()
    pose_pred = pred[:, :6].copy()
    pose_est[0], pose_est[1] = m[0], m[1]
    pose_pred[0], pose_pred[1] = m[0], m[1]
    est[1] = np.concatenate([m[1], m[0]])
    vel = est[1:T - 1, :6] - est[1:T - 1, 6:]
    return (
        pose_est.astype(np.float32),
        pose_pred.astype(np.float32),
        vel.astype(np.float32),
    )


def _toeplitz_banks(west, wpred, wvel):
    """Six 128x128 banks side by side: [W0_e, W1_e, W0_p, W1_p, W0_v, W1_v]."""
    dt_ = np.arange(BLK)[:, None]
    ds_ = np.arange(BLK)[None, :]
    D = ds_ - dt_                        # tap index for current block
    W = np.zeros((BLK, 6 * BLK), dtype=np.float32)
    for j, w in enumerate((west, wpred, wvel)):
        W0 = np.where(D >= 0, w[np.clip(D, 0, NTAPS - 1)], 0.0)
        W1 = np.where(D < 0, w[np.clip(D + NTAPS, 0, NTAPS - 1)], 0.0)
        W[:, (2 * j) * BLK:(2 * j + 1) * BLK] = W0.astype(np.float32)
        W[:, (2 * j + 1) * BLK:(2 * j + 2) * BLK] = W1.astype(np.float32)
    return W


# --------------------------------------------------------------------------- #
# device program
# --------------------------------------------------------------------------- #

def _build_program():
    import concourse.bass as bass
    import concourse.mybir as mybir
    import concourse.tile as tile
    from concourse import bacc

    f32 = mybir.dt.float32
    nc = bacc.Bacc(
        "TRN2",
        target_bir_lowering=False,
        debug=False,
        num_devices=NCORES,
    )
    X_d = nc.dram_tensor("xmat", [BLK, (NB + 1) * 6], f32, kind="ExternalInput").ap()
    W_d = nc.dram_tensor("wmat", [BLK, 6 * BLK], f32, kind="ExternalInput").ap()
    O_d = [
        nc.dram_tensor(n, [BLK, NB * 6], f32, kind="ExternalOutput").ap()
        for n in ("oest", "opred", "ovel")
    ]

    NFREE = NB * 6            # 768 output columns per partition
    NCHUNK = NFREE // 2       # 384 <= 512 (one PSUM bank)

    with tile.TileContext(nc) as tc:
        with (
            tc.tile_pool(name="sb", bufs=1) as sb,
            tc.tile_pool(name="ps", bufs=4, space="PSUM") as ps,
            tc.tile_pool(name="ob", bufs=3) as ob,
        ):
            W = sb.tile([BLK, 6 * BLK], f32, tag="w")
            nc.sync.dma_start(W[:], W_d[:])
            X = sb.tile([BLK, (NB + 1) * 6], f32, tag="x")
            nc.sync.dma_start(X[:], X_d[:])

            for j in range(3):
                O = ob.tile([BLK, NFREE], f32, tag="o")
                for h in range(2):
                    p = ps.tile([BLK, NCHUNK], f32, tag="acc")
                    lo = h * NCHUNK
                    # current blocks: taps 0..δs ; previous blocks: taps δs+1..127
                    nc.tensor.matmul(
                        p[:],
                        W[:, (2 * j) * BLK:(2 * j + 1) * BLK],
                        X[:, 6 + lo: 6 + lo + NCHUNK],
                        start=True,
                        stop=False,
                    )
                    nc.tensor.matmul(
                        p[:],
                        W[:, (2 * j + 1) * BLK:(2 * j + 2) * BLK],
                        X[:, lo: lo + NCHUNK],
                        start=False,
                        stop=True,
                    )
                    nc.vector.tensor_copy(O[:, lo: lo + NCHUNK], p[:])
                nc.sync.dma_start(O_d[j][:], O[:])
    nc.compile()
    return nc


def _get_program():
    if "nc" not in _compiled:
        _compiled["nc"] = _build_program()
    return _compiled["nc"]


# --------------------------------------------------------------------------- #
# entry point
# --------------------------------------------------------------------------- #

def kernel(pose_measurements, process_noise_cov, measure_noise_cov, cov_init,
           _trace=False):
    global LAST_RESULTS
    m32 = np.ascontiguousarray(np.asarray(pose_measurements, dtype=np.float32))
    m = m32.astype(np.float64)
    Qc = np.asarray(process_noise_cov, dtype=np.float64)
    Rc = np.asarray(measure_noise_cov, dtype=np.float64)
    P0 = np.asarray(cov_init, dtype=np.float64)
    assert m32.shape == (T, 6)

    Ks, conv = _riccati(Qc, Rc, P0)
    if conv is None:
        return _host_fallback(m, Ks)
    _, w0, w1, iso_ok = _steady_taps(Ks[-1])
    if not iso_ok:
        return _host_fallback(m, Ks)

    # first device-trusted row: steady gain reached AND full tap window past it
    t0 = conv + 2 + NTAPS + 8

    # FIR taps for the three outputs (see module docstring)
    west = w0
    wpred = np.zeros(NTAPS)
    wpred[1:] = 2 * w0[:NTAPS - 1] - w1[:NTAPS - 1]
    wvel = w0 - w1
    Wbanks = _toeplitz_banks(west, wpred, wvel)

    # per-core overlapped input: rows [start-128, start+16384) as (δt, block, ch)
    padded = np.concatenate([np.zeros((BLK, 6), np.float32), m32], axis=0)
    in_maps = []
    for i in range(NCORES):
        rows = padded[i * TPC: i * TPC + TPC + BLK]          # (16512, 6)
        Xh = np.ascontiguousarray(
            rows.reshape(NB + 1, BLK, 6).transpose(1, 0, 2).reshape(BLK, (NB + 1) * 6)
        )
        in_maps.append({"xmat": Xh, "wmat": Wbanks})

    from concourse.bass_utils import run_bass_kernel_spmd
    nc = _get_program()
    res = run_bass_kernel_spmd(nc, in_maps, core_ids=list(range(NCORES)),
                               trace=_trace)
    LAST_RESULTS = res

    def gather(name):
        parts = [
            res.results[i][name].reshape(BLK, NB, 6).transpose(1, 0, 2).reshape(TPC, 6)
            for i in range(NCORES)
        ]
        return np.concatenate(parts, axis=0)                 # (T, 6), row t

    pose_est = gather("oest")
    pose_pred = gather("opred")
    vel_rows = gather("ovel")                                # row t holds vel[t-1]

    # host patch of the transient (exact fp64 recursion, ~170 steps)
    est, pred = _exact_prefix(m, Ks, t0)
    pose_est[2:t0 + 1] = est[2:t0 + 1, :6].astype(np.float32)
    pose_pred[2:t0 + 1] = pred[2:t0 + 1, :6].astype(np.float32)
    pose_est[0], pose_est[1] = m32[0], m32[1]
    pose_pred[0], pose_pred[1] = m32[0], m32[1]

    vel = vel_rows[1:T - 1].copy()                           # vel[j] <- row j+1
    vel[:t0] = (est[1:t0 + 1, :6] - est[1:t0 + 1, 6:]).astype(np.float32)

    return pose_est, pose_pred, vel


# revision 3
# speedup vs baseline: 1.2676x; 1.2676x over previous
"""Trainium2 Bass kernel for the constant-velocity Kalman filter (T=131072, 6-dof pose).

Reformulation: the covariance/gain recursion is measurement-independent, so the
Kalman gain converges to its steady state K_ss after ~30 steps (Riccati, done on
host in fp64).  From there the filter state is a linear time-invariant recursion
  est_t = A est_{t-1} + K m_t,   A = (I - K H) F,  rho(A) = 0.618,
so est_t equals a 128-tap FIR of the measurements to below fp32 precision
(|A^64 K| ~ 2e-14).  With isotropic noise the taps are scalar per channel, and
all three outputs (pose estimate / prediction / velocity) are scalar FIR
convolutions of the 6-channel measurement stream.

Device work: each of the 8 cores owns 16384 timesteps, laid out as 128 blocks x
128 steps (block-index on the SBUF free dim, in-block time on partitions). The
convolution is two accumulating 128x128 banded-Toeplitz matmuls per output on
the tensor engine (current block + previous block), i.e. overlap-save with the
overlap handled by PSUM accumulation.  The first ~170 rows (Riccati transient)
are patched on the host with the exact sequential recursion in fp64.
"""

import os

import numpy as np

T = 131072
NCORES = 8
TPC = T // NCORES            # timesteps per core
BLK = 128                    # in-block timesteps (partition dim)
NB = TPC // BLK              # 128 blocks per core
NTAPS = 128                  # FIR taps 0..127
RICCATI_CAP = 2048

_compiled = {}
LAST_RESULTS = None


# --------------------------------------------------------------------------- #
# host-side model math (fp64)
# --------------------------------------------------------------------------- #

def _model():
    F = np.zeros((12, 12))
    F[:6, :6] = 2 * np.eye(6)
    F[:6, 6:] = -np.eye(6)
    F[6:, :6] = np.eye(6)
    H = np.zeros((6, 12))
    H[:, :6] = np.eye(6)
    return F, H


def _riccati(Qc, Rc, P0):
    """Run the gain recursion; K_t applies at absolute row t = i + 2."""
    F, H = _model()
    Q = np.zeros((12, 12))
    Q[:6, :6] = Qc
    P = P0.copy()
    Ks = []
    conv = None
    for i in range(RICCATI_CAP):
        Pp = F @ P @ F.T + Q
        S = Pp[:6, :6] + Rc
        K = Pp[:, :6] @ np.linalg.inv(S)
        KH = np.zeros((12, 12))
        KH[:, :6] = K
        P = Pp - KH @ Pp
        Ks.append(K)
        if i > 0 and np.abs(K - Ks[-2]).max() < 1e-13 * (1.0 + np.abs(K).max()):
            conv = i
            break
    return Ks, conv


def _steady_taps(K_ss):
    """Matrix taps A^k K_ss; returns (taps[128,12,6], w0[128], w1[128], iso_ok)."""
    F, H = _model()
    A = (np.eye(12) - K_ss @ H) @ F
    taps = np.zeros((NTAPS, 12, 6))
    Ak = np.eye(12)
    for k in range(NTAPS):
        taps[k] = Ak @ K_ss
        Ak = Ak @ A
    w0 = taps[:, 0, 0].copy()
    w1 = taps[:, 6, 0].copy()
    I6 = np.eye(6)
    iso_err = max(
        np.abs(taps[:, :6, :] - w0[:, None, None] * I6).max(),
        np.abs(taps[:, 6:, :] - w1[:, None, None] * I6).max(),
    )
    tail = np.abs(taps[NTAPS - 1]).max() + np.abs(Ak).max()
    iso_ok = iso_err < 1e-9 and tail < 1e-9
    return taps, w0, w1, iso_ok


def _exact_prefix(m, Ks, t_hi):
    """Exact fp64 sequential recursion for rows 2..t_hi; returns est[t] (12,) array."""
    F, _ = _model()
    est = np.zeros((t_hi + 1, 12))
    pred = np.zeros((t_hi + 1, 12))
    est[1] = np.concatenate([m[1], m[0]])
    e = est[1].copy()
    for t in range(2, t_hi + 1):
        Kt = Ks[min(t - 2, len(Ks) - 1)]
        p = F @ e
        e = p + Kt @ (m[t] - p[:6])
        est[t] = e
        pred[t] = p
    return est, pred


def _host_fallback(m, Ks):
    """Full sequential filter on host (only for pathological covariance inputs)."""
    est, pred = _exact_prefix(m, Ks, T - 1)
    pose_est = est[:, :6].copy()
    pose_pred = pred[:, :6].copy()
    pose_est[0], pose_est[1] = m[0], m[1]
    pose_pred[0], pose_pred[1] = m[0], m[1]
    est[1] = np.concatenate([m[1], m[0]])
    vel = est[1:T - 1, :6] - est[1:T - 1, 6:]
    return (
        pose_est.astype(np.float32),
        pose_pred.astype(np.float32),
        vel.astype(np.float32),
    )


def _toeplitz_banks(west, wpred, wvel):
    """Six 128x128 banks side by side: [W0_e, W1_e, W0_p, W1_p, W0_v, W1_v]."""
    dt_ = np.arange(BLK)[:, None]
    ds_ = np.arange(BLK)[None, :]
    D = ds_ - dt_                        # tap index for current block
    W = np.zeros((BLK, 6 * BLK), dtype=np.float32)
    for j, w in enumerate((west, wpred, wvel)):
        W0 = np.where(D >= 0, w[np.clip(D, 0, NTAPS - 1)], 0.0)
        W1 = np.where(D < 0, w[np.clip(D + NTAPS, 0, NTAPS - 1)], 0.0)
        W[:, (2 * j) * BLK:(2 * j + 1) * BLK] = W0.astype(np.float32)
        W[:, (2 * j + 1) * BLK:(2 * j + 2) * BLK] = W1.astype(np.float32)
    return W


# --------------------------------------------------------------------------- #
# device program
# --------------------------------------------------------------------------- #

def _build_program():
    import concourse.bass as bass
    import concourse.mybir as mybir
    import concourse.tile as tile
    from concourse import bacc

    f32 = mybir.dt.float32
    # float32r: fp32 operands streamed at full PE rate (vs 4 cycles/row for
    # plain fp32) when the moving dim is >=256; PSUM accumulation stays fp32.
    fmm = mybir.dt.float32r if os.environ.get("KF_MM_DTYPE", "f32r") == "f32r" else f32
    nc = bacc.Bacc(
        "TRN2",
        target_bir_lowering=False,
        debug=False,
        num_devices=NCORES,
    )
    X_d = nc.dram_tensor("xmat", [BLK, (NB + 1) * 6], fmm, kind="ExternalInput").ap()
    W_d = nc.dram_tensor("wmat", [BLK, 6 * BLK], fmm, kind="ExternalInput").ap()
    O_d = [
        nc.dram_tensor(n, [BLK, NB * 6], f32, kind="ExternalOutput").ap()
        for n in ("oest", "opred", "ovel")
    ]

    NFREE = NB * 6            # 768 output columns per partition
    NCHUNK = NFREE // 2       # 384 <= 512 (one PSUM bank)

    with tile.TileContext(nc) as tc:
        with (
            tc.tile_pool(name="sb", bufs=1) as sb,
            tc.tile_pool(name="ps", bufs=4, space="PSUM") as ps,
            tc.tile_pool(name="ob", bufs=3) as ob,
        ):
            W = sb.tile([BLK, 6 * BLK], fmm, tag="w")
            nc.sync.dma_start(W[:], W_d[:])
            X = sb.tile([BLK, (NB + 1) * 6], fmm, tag="x")
            nc.sync.dma_start(X[:], X_d[:])

            for j in range(3):
                O = ob.tile([BLK, NFREE], f32, tag="o")
                for h in range(2):
                    p = ps.tile([BLK, NCHUNK], f32, tag="acc")
                    lo = h * NCHUNK
                    # current blocks: taps 0..δs ; previous blocks: taps δs+1..127
                    nc.tensor.matmul(
                        p[:],
                        W[:, (2 * j) * BLK:(2 * j + 1) * BLK],
                        X[:, 6 + lo: 6 + lo + NCHUNK],
                        start=True,
                        stop=False,
                    )
                    nc.tensor.matmul(
                        p[:],
                        W[:, (2 * j + 1) * BLK:(2 * j + 2) * BLK],
                        X[:, lo: lo + NCHUNK],
                        start=False,
                        stop=True,
                    )
                    nc.vector.tensor_copy(O[:, lo: lo + NCHUNK], p[:])
                nc.sync.dma_start(O_d[j][:], O[:])
    nc.compile()
    return nc


def _get_program():
    if "nc" not in _compiled:
        _compiled["nc"] = _build_program()
    return _compiled["nc"]


# --------------------------------------------------------------------------- #
# entry point
# --------------------------------------------------------------------------- #

def kernel(pose_measurements, process_noise_cov, measure_noise_cov, cov_init,
           _trace=False):
    global LAST_RESULTS
    m32 = np.ascontiguousarray(np.asarray(pose_measurements, dtype=np.float32))
    m = m32.astype(np.float64)
    Qc = np.asarray(process_noise_cov, dtype=np.float64)
    Rc = np.asarray(measure_noise_cov, dtype=np.float64)
    P0 = np.asarray(cov_init, dtype=np.float64)
    assert m32.shape == (T, 6)

    Ks, conv = _riccati(Qc, Rc, P0)
    if conv is None:
        return _host_fallback(m, Ks)
    _, w0, w1, iso_ok = _steady_taps(Ks[-1])
    if not iso_ok:
        return _host_fallback(m, Ks)

    # first device-trusted row: steady gain reached AND full tap window past it
    t0 = conv + 2 + NTAPS + 8

    # FIR taps for the three outputs (see module docstring)
    west = w0
    wpred = np.zeros(NTAPS)
    wpred[1:] = 2 * w0[:NTAPS - 1] - w1[:NTAPS - 1]
    wvel = w0 - w1
    Wbanks = _toeplitz_banks(west, wpred, wvel)

    # per-core overlapped input: rows [start-128, start+16384) as (δt, block, ch)
    padded = np.concatenate([np.zeros((BLK, 6), np.float32), m32], axis=0)
    in_maps = []
    for i in range(NCORES):
        rows = padded[i * TPC: i * TPC + TPC + BLK]          # (16512, 6)
        Xh = np.ascontiguousarray(
            rows.reshape(NB + 1, BLK, 6).transpose(1, 0, 2).reshape(BLK, (NB + 1) * 6)
        )
        in_maps.append({"xmat": Xh, "wmat": Wbanks})

    from concourse.bass_utils import run_bass_kernel_spmd
    nc = _get_program()
    res = run_bass_kernel_spmd(nc, in_maps, core_ids=list(range(NCORES)),
                               trace=_trace)
    LAST_RESULTS = res

    def gather(name):
        parts = [
            res.results[i][name].reshape(BLK, NB, 6).transpose(1, 0, 2).reshape(TPC, 6)
            for i in range(NCORES)
        ]
        return np.concatenate(parts, axis=0)                 # (T, 6), row t

    pose_est = gather("oest")
    pose_pred = gather("opred")
    vel_rows = gather("ovel")                                # row t holds vel[t-1]

    # host patch of the transient (exact fp64 recursion, ~170 steps)
    est, pred = _exact_prefix(m, Ks, t0)
    pose_est[2:t0 + 1] = est[2:t0 + 1, :6].astype(np.float32)
    pose_pred[2:t0 + 1] = pred[2:t0 + 1, :6].astype(np.float32)
    pose_est[0], pose_est[1] = m32[0], m32[1]
    pose_pred[0], pose_pred[1] = m32[0], m32[1]

    vel = vel_rows[1:T - 1].copy()                           # vel[j] <- row j+1
    vel[:t0] = (est[1:t0 + 1, :6] - est[1:t0 + 1, 6:]).astype(np.float32)

    return pose_est, pose_pred, vel
